# revision 30
# baseline (speedup 1.0000x reference)
"""Trainium2 Bass kernel for causal self-attention (muP scaling).

Full-input contract: kernel(**inputs) takes the complete tensors and returns
the complete [B, T, C] output. Internally the work is split over 8 NeuronCores
as (batch b = core//2) x (head-group g = core%2, 8 heads each):

  - each core computes q,k,v for its batch restricted to its 8 heads,
    runs causal attention for those heads, and multiplies by the matching
    512-row slice of w_proj, producing a partial [T, C] output.
  - the host sums the two partials per batch and adds b_proj. No on-device
    collectives are needed.

Layout trick: the host passes x[b].T (i.e. [C, T]) so that
  - qT,kT ([dim, t]) come from matmuls with the weight slice as the
    stationary operand and xT as the moving operand,
  - v ([t, dim]) comes from matmuls with xT tiles as the stationary operand,
so no on-chip transposes are needed anywhere.

Attention runs per head PAIR: the even head lives at SBUF partitions 0:64
and the odd head at 64:128 of the qkT tiles, so the two K=64 score matmuls
occupy disjoint PE row-groups (concurrent in the systolic array) and write
the two banks of one [128, 1024] PSUM tile, which a single ScalarE exp
drains (2-segment strided AP; muP scale 1/64 folded into the activation
scale; no max-subtraction - logits are ~N(0, 0.13) so exp cannot overflow).
Causal masking is a 0/1 upper-triangular multiply on diagonal-crossing
tiles only; fully-invalid tiles are never computed. attT-out[d, tq]
accumulates v_aug.T @ expT where v_aug carries an appended ones column, so
row 64 of the accumulator is the softmax denominator for free.
Normalization: reciprocal of that row, partition-broadcast on GpSimd, one
fused multiply while copying PSUM->SBUF. The normalized attention output
lands directly in [c, t] layout - the stationary-operand layout the final
projection wants. Attention blocks iterate tq-block-outer so each finished
tq column group's output projection interleaves with the next block's
(ScalarE-paced) attention. Activations ride bf16 (inputs pre-cast on the
host); measured end-to-end error vs the fp32 reference is ~4e-3 relative.
"""

import sys

if "/opt/trn_rl_repo" not in sys.path:
    sys.path.insert(0, "/opt/trn_rl_repo")

import numpy as np
import ml_dtypes

import concourse.bass as bass
import concourse.mybir as mybir
import concourse.tile as tile
from concourse import bacc
from concourse.bass_utils import run_bass_kernel_spmd
from concourse.masks import make_upper_triangular

# Problem shape (hardcoded per contract).
B, T, C, H = 4, 2048, 1024, 16
HD = C // H            # 64
N_CORES = 8
HG = H // 2            # 8 heads per core
GC = HG * HD           # 512 columns of q/k/v per core
P = 128                # SBUF partitions
CT = C // P            # 8 contraction tiles over C
TT = T // P            # 16 time tiles of 128
QB = 4                 # tq blocks
QW = T // QB           # 512 wide
KT = T // P            # 16 tk tiles

_bf16np = ml_dtypes.bfloat16
_fp8np = ml_dtypes.float8_e4m3
F32 = mybir.dt.float32
F32R = mybir.dt.float32r
BF16 = mybir.dt.bfloat16
FP8 = mybir.dt.float8e4

_COMPILED = None


def _r(ap):
    """Reinterpret an fp32 AP as float32r for full-rate PE matmuls."""
    return ap.bitcast(F32R)


def _build_nc(reps=1, phases=(1, 2, 3), p2mode="full", pipeline=False, all_bf16=True, exp_split=False, qk_fp8=True, fused=True):
    nc = bacc.Bacc("TRN2", target_bir_lowering=False, debug=False,
                   num_devices=N_CORES)

    adt = BF16 if all_bf16 else F32
    xT = nc.dram_tensor("xT", [C, T], adt, kind="ExternalInput").ap()
    if qk_fp8:
        xT8 = nc.dram_tensor("xT8", [C, T], FP8, kind="ExternalInput").ap()
        w_qk = nc.dram_tensor("w_qk8", [C, 2 * GC], FP8, kind="ExternalInput").ap()
    else:
        xT8 = None
        w_qk = nc.dram_tensor("w_qk", [C, 2 * GC], adt, kind="ExternalInput").ap()
    w_v = nc.dram_tensor("w_v", [C, GC], adt, kind="ExternalInput").ap()
    b_qk = nc.dram_tensor("b_qk", [2 * GC], F32, kind="ExternalInput").ap()
    b_v = nc.dram_tensor("b_v", [GC], F32, kind="ExternalInput").ap()
    w_pr = nc.dram_tensor("w_pr", [GC, C], BF16, kind="ExternalInput").ap()
    y = nc.dram_tensor("y", [T, C], F32, kind="ExternalOutput").ap()

    with tile.TileContext(nc) as tc:
        for _ in range(reps):
            if fused:
                assert qk_fp8
                _emit_fused(nc, tc, xT, w_qk, w_v, b_qk, b_v, w_pr, y, xT8)
            else:
                _emit(nc, tc, xT, w_qk, w_v, b_qk, b_v, w_pr, y, phases=phases, p2mode=p2mode, pipeline=pipeline, all_bf16=all_bf16, exp_split=exp_split, xT8=xT8)
    nc.finalize()
    return nc


def _emit_fused(nc, tc, xT, w_qk, w_v, b_qk, b_v, w_pr, y, xT8, expp_bufs=20):
    """Single software-pipelined pass over tq stages.

    Stage s emits: qk-projection for tq block s (fp8 DoubleRow, with the
    previous attention block's AV matmuls interleaved), v-projection for the
    matching four time tiles, the output projection of stage s-1, then the
    four attention head-pair blocks of tq block s (scores paced by ScalarE
    exp; AV of each block drains during the next block's scores).  This keeps
    ScalarE exp running from ~10% of the program instead of idling through a
    serial QKV phase, and PSUM rotates through exactly 8 banks:
    2x[128,512] (qk/v/proj shared) + 2x[128,1024] scores + 2x[128,512] acc.
    """
    from contextlib import ExitStack

    ctx = ExitStack()
    with ctx:
        persist = ctx.enter_context(tc.tile_pool(name="persist", bufs=1))

        tri = persist.tile([P, P], BF16, tag="tri")     # 0/1, 1 iff j >= i
        make_upper_triangular(nc, tri[:, :], val=1.0, diag=True)

        bqk_sb = persist.tile([P, CT], F32, tag="bqk")  # [128, 8] col jt
        nc.sync.dma_start(
            out=bqk_sb[:, :],
            in_=bass.AP(tensor=b_qk.tensor, offset=0, ap=[[1, P], [P, CT]]),
        )
        bv_sb = persist.tile([P, GC], F32, tag="bv")
        nc.gpsimd.dma_start(
            out=bv_sb[:, :],
            in_=bass.AP(tensor=b_v.tensor, offset=0, ap=[[0, P], [1, GC]]),
        )

        qkT = [persist.tile([P, T], BF16, name=f"qkT{j}", tag=f"qkT{j}")
               for j in range(CT)]
        v_sb = [persist.tile([P, HG, HD + 1], BF16, name=f"v{t}", tag=f"v{t}")
                for t in range(TT)]
        att = [persist.tile([P, T], BF16, name=f"att{j}", tag=f"att{j}")
               for j in range(CT // 2)]
        wpr = [persist.tile([P, C], BF16, name=f"wpr{j}", tag=f"wpr{j}")
               for j in range(CT // 2)]

        ap_ = ctx.enter_context(tc.tile_pool(name="apool", bufs=1))
        x8s = [ap_.tile([P, 2, T], FP8, name=f"x8_{j}", tag=f"x8_{j}")
               for j in range(CT // 2)]
        w8s = [ap_.tile([P, 2, 2 * GC], FP8, name=f"w8_{j}", tag=f"w8_{j}")
               for j in range(CT // 2)]
        xts = [ap_.tile([P, T], BF16, name=f"xT{ct}", tag=f"xT{ct}")
               for ct in range(CT)]
        wvts = [ap_.tile([P, GC], BF16, name=f"wv{ct}", tag=f"wv{ct}")
                for ct in range(CT)]

        # All input DMAs ride the SP queue in need-order (compute queues must
        # stay DMA-free: a queued DMA head-of-line blocks the engine's
        # sequencer).  Block-0 operands first, then the rest merged into a
        # single transfer per tile.
        for j in range(CT // 2):
            nc.sync.dma_start(out=x8s[j][:, :, 0:QW], in_=bass.AP(
                tensor=xT8.tensor, offset=2 * j * P * T,
                ap=[[T, P], [P * T, 2], [1, QW]]))
            nc.sync.dma_start(out=w8s[j][:, :, :], in_=bass.AP(
                tensor=w_qk.tensor, offset=2 * j * P * 2 * GC,
                ap=[[2 * GC, P], [P * 2 * GC, 2], [1, 2 * GC]]))
        for ct in range(CT):
            nc.sync.dma_start(out=xts[ct][:, 0:QW],
                              in_=xT[ct * P:(ct + 1) * P, 0:QW])
            nc.sync.dma_start(out=wvts[ct][:, :],
                              in_=w_v[ct * P:(ct + 1) * P, :])
        for j in range(CT // 2):
            nc.sync.dma_start(out=x8s[j][:, :, QW:T], in_=bass.AP(
                tensor=xT8.tensor, offset=2 * j * P * T + QW,
                ap=[[T, P], [P * T, 2], [1, T - QW]]))
        for ct in range(CT):
            nc.sync.dma_start(out=xts[ct][:, QW:T],
                              in_=xT[ct * P:(ct + 1) * P, QW:T])
        for ct in range(CT // 2):
            nc.sync.dma_start(out=wpr[ct][:, :],
                              in_=w_pr[ct * P:(ct + 1) * P, :])

        p1 = ctx.enter_context(tc.tile_pool(name="p1", bufs=2, space="PSUM"))
        pss = ctx.enter_context(tc.tile_pool(name="ps_s", bufs=2, space="PSUM"))
        pso = ctx.enter_context(tc.tile_pool(name="ps_o", bufs=2, space="PSUM"))
        expp = ctx.enter_context(tc.tile_pool(name="expp", bufs=expp_bufs))
        nrm = ctx.enter_context(tc.tile_pool(name="nrm", bufs=4))
        yp = ctx.enter_context(tc.tile_pool(name="ysb", bufs=2))

        def emit_scores(hp, q0, kt, off, crossing):
            n = QW - off
            qT_t, kT_t = qkT[hp], qkT[CT // 2 + hp]
            ex = expp.tile([P, 2 * QW], BF16, tag="exp")
            ps = pss.tile([P, 2 * QW], F32, tag="scores")
            nc.tensor.matmul(
                ps[:, 0:n],
                kT_t[0:HD, kt * P:(kt + 1) * P],
                qT_t[0:HD, q0 + off:q0 + QW],
                start=True, stop=True,
            )
            nc.tensor.matmul(
                ps[:, QW:QW + n],
                kT_t[HD:P, kt * P:(kt + 1) * P],
                qT_t[HD:P, q0 + off:q0 + QW],
                start=True, stop=True,
            )
            # one exp over both heads: 2-segment strided view
            ps2 = ps[:, :].rearrange("p (s q) -> p s q", s=2)
            ex2 = ex[:, :].rearrange("p (s q) -> p s q", s=2)
            nc.scalar.activation(
                out=ex2[:, :, 0:n], in_=ps2[:, :, 0:n],
                func=mybir.ActivationFunctionType.Exp,
                scale=1.0 / HD,
            )
            if crossing:
                nc.vector.tensor_mul(
                    out=ex[:, 0:P], in0=ex[:, 0:P], in1=tri[:, :])
                nc.vector.tensor_mul(
                    out=ex[:, QW:QW + P], in0=ex[:, QW:QW + P], in1=tri[:, :])
            return ex

        def emit_av(st, i):
            (hp, q0, accs, exps) = st
            kt, off, n, ex = exps[i]
            last = i == len(exps) - 1
            nc.tensor.matmul(
                accs[0][0:HD + 1, off:QW],
                v_sb[kt][:, 2 * hp, :],
                ex[:, 0:n],
                start=(i == 0), stop=last,
                skip_group_check=True,
            )
            nc.tensor.matmul(
                accs[1][0:HD + 1, off:QW],
                v_sb[kt][:, 2 * hp + 1, :],
                ex[:, QW:QW + n],
                start=(i == 0), stop=last,
                skip_group_check=True,
            )

        def emit_norm(st):
            (hp, q0, accs, exps) = st
            for half, acc in ((0, accs[0]), (1, accs[1])):
                r0 = half * HD
                rec = nrm.tile([P, QW], F32, tag="rec")
                nc.vector.reciprocal(out=rec[0:1, :], in_=acc[HD:HD + 1, :])
                bc = nrm.tile([P, QW], F32, tag="bc")
                nc.gpsimd.partition_broadcast(
                    bc[0:HD, :], rec[0:1, :], channels=HD)
                nc.vector.tensor_mul(
                    out=att[hp][r0:r0 + HD, q0:q0 + QW],
                    in0=acc[0:HD, :],
                    in1=bc[0:HD, :],
                )

        ysb_live = {}

        def emit_proj_unit(tt, nb):
            if nb == 0:
                ysb_live[tt] = yp.tile([P, C], F32, name=f"ysb{tt}", tag="y")
            ysb = ysb_live[tt]
            ps = p1.tile([P, QW], F32, tag="p1")
            for ct in range(CT // 2):
                nc.tensor.matmul(
                    ps[:, :],
                    att[ct][:, tt * P:(tt + 1) * P],
                    wpr[ct][:, nb * QW:(nb + 1) * QW],
                    start=(ct == 0), stop=(ct == CT // 2 - 1),
                )
            nc.vector.tensor_copy(
                out=ysb[:, nb * QW:(nb + 1) * QW], in_=ps[:, :])
            if nb == 1:
                nc.gpsimd.dma_start(out=y[tt * P:(tt + 1) * P, :], in_=ysb[:, :])
                del ysb_live[tt]

        def qk_unit(s, jt):
            ps = p1.tile([P, QW], F32, name=f"pqk{s}_{jt}", tag="p1")
            for j in range(CT // 2):
                nc.tensor.matmul(
                    ps[:, :],
                    w8s[j][:, :, jt * P:(jt + 1) * P],
                    x8s[j][:, :, s * QW:(s + 1) * QW],
                    start=(j == 0), stop=(j == CT // 2 - 1),
                    perf_mode=mybir.MatmulPerfMode.DoubleRow,
                )
            nc.scalar.activation(
                out=qkT[jt][:, s * QW:(s + 1) * QW],
                in_=ps[:, :],
                func=mybir.ActivationFunctionType.Identity,
                bias=bqk_sb[:, jt:jt + 1],
            )

        def v_unit(s, i):
            tt = s * 4 + i
            ps = p1.tile([P, GC], F32, name=f"pv{tt}", tag="p1")
            for ct in range(CT):
                nc.tensor.matmul(
                    ps[:, :],
                    xts[ct][:, tt * P:(tt + 1) * P],
                    wvts[ct][:, :],
                    start=(ct == 0), stop=(ct == CT - 1),
                )
            nc.vector.tensor_add(
                out=v_sb[tt][:, :, 0:HD],
                in0=ps[:, :].rearrange("p (h e) -> p h e", e=HD),
                in1=bv_sb[:, :].rearrange("p (h e) -> p h e", e=HD),
            )
            nc.vector.memset(v_sb[tt][:, :, HD:HD + 1], 1.0)

        # jt order inside a stage: the head-pair hp scores need qkT[hp] (q)
        # and qkT[4+hp] (k), so emit k/q pairs in hp order — the first
        # attention block can start after two units instead of five.
        JT_ORDER = [0, 4, 1, 5, 2, 6, 3, 7]

        # ---------------- flat software-pipelined schedule ----------------
        # One pass over the 16 attention blocks.  The qk/v projections of
        # stage s+1 and the output projection of stage s-1 are "filler" PE
        # units drained into this stage's score-tile slots at a rate that
        # lands them all by stage end, keeping ScalarE exp saturated while
        # the PE works ahead.  Fillers run before the paced pend-AV drain of
        # a slot so a just-emitted v tile is in place for its AV consumer.
        filler_q = []   # qk/v units: must land by end of current stage
        lazy_q = []     # proj units: paced across the remaining program,
                        # shifting their PE work into the late, exp-bound
                        # stages where the PE otherwise idles
        pend = None

        def drain_filler(k):
            for _ in range(min(k, len(filler_q))):
                kind, a, b = filler_q.pop(0)
                if kind == "qk":
                    qk_unit(a, b)
                elif kind == "v":
                    v_unit(a, b)
                else:
                    emit_proj_unit(a, b)

        # prologue: just enough of qk block 0 for the first attention block
        qk_unit(0, 0)
        qk_unit(0, 4)
        filler_q += [("qk", 0, 1), ("qk", 0, 5), ("v", 0, 0), ("v", 0, 1),
                     ("v", 0, 2), ("v", 0, 3), ("qk", 0, 2), ("qk", 0, 6),
                     ("qk", 0, 3), ("qk", 0, 7)]

        total_slots = sum((HG // 2) * (4 * s + 4) for s in range(QB))
        gslot = 0
        for s in range(QB):
            q0 = s * QW
            stage_slots = (HG // 2) * (4 * s + 4)
            slot = 0
            for hp in range(HG // 2):
                if s + 1 < QB and hp == (2 if s == 0 else 1):
                    # queue next stage's projections; the drain rate lands
                    # them before block (0, s+1) needs qkT/v.
                    filler_q += [("qk", s + 1, jt) for jt in JT_ORDER]
                    filler_q += [("v", s + 1, i) for i in range(4)]
                tiles = [(kt, 0, False) for kt in range(4 * s)]
                tiles += [(4 * s + a, P * a, True) for a in range(4)]
                acc_e = pso.tile([P, QW], F32, name=f"acc_e{hp}_{s}", tag="acc")
                acc_o = pso.tile([P, QW], F32, name=f"acc_o{hp}_{s}", tag="acc")
                exps = []
                last_blk = (s == QB - 1) and (hp == HG // 2 - 1)
                np_prev = len(pend[3]) if pend is not None else 0
                st = (hp, q0, (acc_e, acc_o), exps)
                for i, (kt, off, crossing) in enumerate(tiles):
                    ex = emit_scores(hp, q0, kt, off, crossing)
                    exps.append((kt, off, QW - off, ex))
                    slot += 1
                    gslot += 1
                    rem = stage_slots - slot + 1
                    drain_filler(-(-len(filler_q) // max(1, rem)))
                    grem = total_slots - gslot + 1
                    for _ in range(min(-(-len(lazy_q) // max(1, grem)),
                                       len(lazy_q))):
                        emit_proj_unit(*lazy_q.pop(0))
                    if pend is not None:
                        lo = i * np_prev // len(tiles)
                        hi = (i + 1) * np_prev // len(tiles)
                        for j in range(lo, hi):
                            emit_av(pend, j)
                    if last_blk and i > 0:
                        # final block: 1-tile-lookahead AV so the tail is
                        # one AV pair + norm + proj, not a whole block drain
                        emit_av(st, i - 1)
                if pend is not None:
                    emit_norm(pend)
                    if pend[1] == q0 - QW and pend[0] == HG // 2 - 1:
                        # previous stage fully normalized: queue its projection
                        ps_ = (s - 1) * 4
                        lazy_q += [(tt, nb)
                                   for tt in range(ps_, ps_ + 4)
                                   for nb in range(2)]
                pend = None if last_blk else st
                if last_blk:
                    emit_av(st, len(tiles) - 1)
                    emit_norm(st)
        drain_filler(len(filler_q))
        for tt, nb in lazy_q:
            emit_proj_unit(tt, nb)
        for tt in range((QB - 1) * 4, QB * 4):
            emit_proj_unit(tt, 0)
            emit_proj_unit(tt, 1)


def _emit(nc, tc, xT, w_qk, w_v, b_qk, b_v, w_pr, y, phases=(1, 2, 3), p2mode="full", pipeline=False, all_bf16=True, exp_split=False, xT8=None):
    from contextlib import ExitStack

    ctx = ExitStack()
    with ctx:
        persist = ctx.enter_context(tc.tile_pool(name="persist", bufs=1))

        # ---- constants -------------------------------------------------
        tri = persist.tile([P, P], BF16, tag="tri")     # 0/1, 1 iff j >= i
        make_upper_triangular(nc, tri[:, :], val=1.0, diag=True)

        bqk_sb = persist.tile([P, CT], F32, tag="bqk")  # [128, 8] col jt
        nc.sync.dma_start(
            out=bqk_sb[:, :],
            in_=bass.AP(tensor=b_qk.tensor, offset=0, ap=[[1, P], [P, CT]]),
        )
        bv_sb = persist.tile([P, GC], F32, tag="bv")
        nc.gpsimd.dma_start(
            out=bv_sb[:, :],
            in_=bass.AP(tensor=b_v.tensor, offset=0, ap=[[0, P], [1, GC]]),
        )

        # ---- persistent activation buffers ----------------------------
        mdt = BF16 if all_bf16 else F32R
        qkT = [persist.tile([P, T], mdt, name=f"qkT{j}", tag=f"qkT{j}") for j in range(CT)]
        v_sb = [persist.tile([P, HG, HD + 1], BF16, name=f"v{t}", tag=f"v{t}")
                for t in range(TT)]

        # ================= phase 1: qkv projections ====================
        with tc.tile_pool(name="xT", bufs=1) as xp:
            xts = [xp.tile([P, T], mdt, name=f"xT{ct}", tag=f"xT{ct}")
                   for ct in range(CT)]

            qk_fp8 = xT8 is not None
            with tc.tile_pool(name="wqk", bufs=1) as wp, \
                 tc.tile_pool(name="ps1", bufs=8, space="PSUM") as ps1:
                if qk_fp8:
                    # fp8 DoubleRow: pack ct tile pairs (2j, 2j+1) along a
                    # size-2 free dim; one matmul contracts 256 rows.
                    x8s, w8s = [], []
                    for j in range(CT // 2):
                        x8 = xp.tile([P, 2, T], FP8, name=f"x8_{j}", tag=f"x8_{j}")
                        nc.sync.dma_start(out=x8[:, :, :], in_=bass.AP(
                            tensor=xT8.tensor, offset=2 * j * P * T,
                            ap=[[T, P], [P * T, 2], [1, T]]))
                        w8 = wp.tile([P, 2, 2 * GC], FP8, name=f"w8_{j}", tag=f"w8_{j}")
                        nc.sync.dma_start(out=w8[:, :, :], in_=bass.AP(
                            tensor=w_qk.tensor, offset=2 * j * P * 2 * GC,
                            ap=[[2 * GC, P], [P * 2 * GC, 2], [1, 2 * GC]]))
                        x8s.append(x8)
                        w8s.append(w8)
                    for ct in range(CT):
                        nc.sync.dma_start(out=xts[ct][:, :],
                                          in_=xT[ct * P:(ct + 1) * P, :])
                else:
                    wts = [wp.tile([P, 2 * GC], mdt, name=f"wqk{ct}", tag=f"wqk{ct}")
                           for ct in range(CT)]
                    # interleave x/w loads so the first accumulation step's
                    # operands (x0, w0) land before the tail of either stream
                    for ct in range(CT):
                        nc.sync.dma_start(out=xts[ct][:, :],
                                          in_=xT[ct * P:(ct + 1) * P, :] if all_bf16
                                          else xT[ct * P:(ct + 1) * P, :].bitcast(F32R))
                        nc.sync.dma_start(out=wts[ct][:, :],
                                          in_=w_qk[ct * P:(ct + 1) * P, :] if all_bf16
                                          else w_qk[ct * P:(ct + 1) * P, :].bitcast(F32R))
                for jt in range(CT if 1 in phases else 0):
                    # ct-outer so the first matmuls only need tile ct=0 loaded
                    pss_ = [ps1.tile([P, QW], F32, name=f"ps1_{jt}_{tb}", tag="ps1")
                            for tb in range(QB)]
                    if qk_fp8:
                        for j in range(CT // 2):
                            for tb in range(QB):
                                nc.tensor.matmul(
                                    pss_[tb][:, :],
                                    w8s[j][:, :, jt * P:(jt + 1) * P],
                                    x8s[j][:, :, tb * QW:(tb + 1) * QW],
                                    start=(j == 0), stop=(j == CT // 2 - 1),
                                    perf_mode=mybir.MatmulPerfMode.DoubleRow,
                                )
                    else:
                        for ct in range(CT):
                            for tb in range(QB):
                                nc.tensor.matmul(
                                    pss_[tb][:, :],
                                    wts[ct][:, jt * P:(jt + 1) * P],
                                    xts[ct][:, tb * QW:(tb + 1) * QW],
                                    start=(ct == 0), stop=(ct == CT - 1),
                                )
                    for tb in range(QB):
                        # bias-add on ScalarE: ACT is otherwise idle during
                        # the projection phase, freeing DVE for attention
                        nc.scalar.activation(
                            out=qkT[jt][:, tb * QW:(tb + 1) * QW],
                            in_=pss_[tb][:, :],
                            func=mybir.ActivationFunctionType.Identity,
                            bias=bqk_sb[:, jt:jt + 1],
                        )

            with tc.tile_pool(name="wv", bufs=1) as wvp, \
                 tc.tile_pool(name="ps1v", bufs=8, space="PSUM") as ps1v:
                wvts = []
                for ct in range(CT):
                    wvt = wvp.tile([P, GC], mdt, name=f"wv{ct}", tag=f"wv{ct}")
                    nc.sync.dma_start(out=wvt[:, :],
                                      in_=w_v[ct * P:(ct + 1) * P, :] if all_bf16
                                      else w_v[ct * P:(ct + 1) * P, :].bitcast(F32R))
                    wvts.append(wvt)
                for tg in range(TT // 4 if 1 in phases else 0):
                    pss_ = [ps1v.tile([P, GC], F32, name=f"ps1v_{tg}_{i}", tag="ps1v")
                            for i in range(4)]
                    for ct in range(CT):
                        for i in range(4):
                            tt = tg * 4 + i
                            nc.tensor.matmul(
                                pss_[i][:, :],
                                xts[ct][:, tt * P:(tt + 1) * P],
                                wvts[ct][:, :],
                                start=(ct == 0), stop=(ct == CT - 1),
                            )
                    for i in range(4):
                        tt = tg * 4 + i
                        nc.vector.tensor_add(
                            out=v_sb[tt][:, :, 0:HD],
                            in0=pss_[i][:, :].rearrange("p (h e) -> p h e", e=HD),
                            in1=bv_sb[:, :].rearrange("p (h e) -> p h e", e=HD),
                        )
                        nc.vector.memset(v_sb[tt][:, :, HD:HD + 1], 1.0)

        # ================= phase 2: attention ==========================
        # Opened after the xT pool closes so its SBUF space is reused.
        ph23 = ctx.enter_context(tc.tile_pool(name="ph23", bufs=1))
        att = [ph23.tile([P, T], BF16, name=f"att{j}", tag=f"att{j}") for j in range(CT // 2)]
        if p2mode in ("av_only", "scores_av"):
            dummy_ex = ph23.tile([P, 2 * QW], BF16, tag="dummy_ex")
            nc.vector.memset(dummy_ex[:, :], 0.5)
        if p2mode != "full":
            for j in range(CT // 2):
                nc.vector.memset(att[j][:, :], 0.01)
        wpr = [ph23.tile([P, C], BF16, name=f"wpr{j}", tag=f"wpr{j}") for j in range(CT // 2)]
        for ct in range(CT // 2):
            nc.sync.dma_start(out=wpr[ct][:, :], in_=w_pr[ct * P:(ct + 1) * P, :])

        do_scores = p2mode in ("full", "scores_only", "scores_exp", "scores_av")
        do_exp = p2mode in ("full", "scores_exp")
        do_av = p2mode in ("full", "av_only", "scores_av")
        do_norm = p2mode == "full"

        with tc.tile_pool(name="expp", bufs=20) as expp, \
             tc.tile_pool(name="nrm", bufs=4) as nrm, \
             tc.tile_pool(name="ysb", bufs=3) as yp, \
             tc.tile_pool(name="ps_s", bufs=2, space="PSUM") as pss, \
             tc.tile_pool(name="ps_o", bufs=2, space="PSUM") as pso, \
             tc.tile_pool(name="ps3", bufs=2, space="PSUM") as ps3:
            # Head PAIRS: even head at partitions 0:64, odd at 64:128 of the
            # qkT tiles. The two score matmuls use disjoint PE row-groups and
            # run concurrently; their outputs land in the two banks of one
            # [128, 1024] PSUM tile so a single ACT exp drains both.
            #
            # Software pipeline across (pair, block) iterations: the AV
            # matmuls of block k-1 are interleaved tile-by-tile with the
            # score matmuls of block k, so the PE never sits waiting for
            # ScalarE to finish the exps of the block it just scored.
            blocks = []
            if 2 in phases:
                for qb in range(QB):
                    for hp in range(HG // 2):
                        tiles = [(kt, 0, False) for kt in range(4 * qb)]
                        tiles += [(4 * qb + a, P * a, True) for a in range(4)]
                        blocks.append((hp, qb, tiles))

            def emit_scores(hp, q0, kt, off, crossing):
                n = QW - off
                qT_t, kT_t = qkT[hp], qkT[CT // 2 + hp]
                ex = expp.tile([P, 2 * QW], BF16, tag="exp")
                if not do_scores:
                    return dummy_ex if do_av else ex
                ps = pss.tile([P, 2 * QW], F32, tag="scores")
                nc.tensor.matmul(
                    ps[:, 0:n],
                    kT_t[0:HD, kt * P:(kt + 1) * P],
                    qT_t[0:HD, q0 + off:q0 + QW],
                    start=True, stop=True,
                )
                nc.tensor.matmul(
                    ps[:, QW:QW + n],
                    kT_t[HD:P, kt * P:(kt + 1) * P],
                    qT_t[HD:P, q0 + off:q0 + QW],
                    start=True, stop=True,
                )
                if do_exp:
                    if exp_split:
                        nc.scalar.activation(
                            out=ex[:, 0:n], in_=ps[:, 0:n],
                            func=mybir.ActivationFunctionType.Exp,
                            scale=1.0 / HD,
                        )
                        nc.scalar.activation(
                            out=ex[:, QW:QW + n], in_=ps[:, QW:QW + n],
                            func=mybir.ActivationFunctionType.Exp,
                            scale=1.0 / HD,
                        )
                    else:
                        # one exp over both heads: 2-segment strided view
                        ps2 = ps[:, :].rearrange("p (s q) -> p s q", s=2)
                        ex2 = ex[:, :].rearrange("p (s q) -> p s q", s=2)
                        nc.scalar.activation(
                            out=ex2[:, :, 0:n], in_=ps2[:, :, 0:n],
                            func=mybir.ActivationFunctionType.Exp,
                            scale=1.0 / HD,
                        )
                    if crossing:
                        # diagonal-crossing tile: triangle on cols 0:128
                        nc.vector.tensor_mul(
                            out=ex[:, 0:P], in0=ex[:, 0:P], in1=tri[:, :])
                        nc.vector.tensor_mul(
                            out=ex[:, QW:QW + P], in0=ex[:, QW:QW + P],
                            in1=tri[:, :])
                else:
                    # timing diagnostics: tiny consumer so the score matmuls
                    # aren't dead code
                    nc.vector.tensor_copy(out=ex[:, 0:2].bitcast(F32),
                                          in_=ps[:, 0:1])
                    if do_av:
                        ex = dummy_ex
                return ex

            def emit_av(st, i):
                (hp, q0, accs, exps) = st
                kt, off, n, ex = exps[i]
                last = i == len(exps) - 1
                nc.tensor.matmul(
                    accs[0][0:HD + 1, off:QW],
                    v_sb[kt][:, 2 * hp, :],
                    ex[:, 0:n],
                    start=(i == 0), stop=last,
                    skip_group_check=True,
                )
                nc.tensor.matmul(
                    accs[1][0:HD + 1, off:QW],
                    v_sb[kt][:, 2 * hp + 1, :],
                    ex[:, QW:QW + n],
                    start=(i == 0), stop=last,
                    skip_group_check=True,
                )

            def emit_norm(st):
                (hp, q0, accs, exps) = st
                for half, acc in ((0, accs[0]), (1, accs[1])):
                    r0 = half * HD
                    if do_norm:
                        rec = nrm.tile([P, QW], F32, tag="rec")
                        nc.vector.reciprocal(out=rec[0:1, :],
                                             in_=acc[HD:HD + 1, :])
                        bc = nrm.tile([P, QW], F32, tag="bc")
                        nc.gpsimd.partition_broadcast(
                            bc[0:HD, :], rec[0:1, :], channels=HD)
                        nc.vector.tensor_mul(
                            out=att[hp][r0:r0 + HD, q0:q0 + QW],
                            in0=acc[0:HD, :],
                            in1=bc[0:HD, :],
                        )
                    else:
                        nc.vector.tensor_copy(
                            out=att[hp][r0:r0 + HD, q0:q0 + QW],
                            in_=acc[0:HD, :])

            def emit_proj_group(tts):
                if 3 not in phases:
                    return
                for tt in tts:
                    ysb = yp.tile([P, C], F32, tag="y")
                    for nb in range(2):
                        ps = ps3.tile([P, QW], F32, tag="ps3")
                        for ct in range(CT // 2):
                            nc.tensor.matmul(
                                ps[:, :],
                                att[ct][:, tt * P:(tt + 1) * P],
                                wpr[ct][:, nb * QW:(nb + 1) * QW],
                                start=(ct == 0), stop=(ct == CT // 2 - 1),
                            )
                        nc.vector.tensor_copy(
                            out=ysb[:, nb * QW:(nb + 1) * QW], in_=ps[:, :])
                    nc.sync.dma_start(out=y[tt * P:(tt + 1) * P, :], in_=ysb[:, :])

            pend = None  # previous block waiting for its AV matmuls
            done_qb = -1
            for hp, qb, tiles in blocks:
                if qb != done_qb and done_qb >= 0:
                    # tq columns of the finished qb group are final in att:
                    # overlap their output projection with this qb's attention
                    if pend is not None and do_av:
                        for j in range(len(pend[3])):
                            emit_av(pend, j)
                        emit_norm(pend)
                        pend = None
                    emit_proj_group(range(done_qb * 4, done_qb * 4 + 4))
                done_qb = qb
                q0 = qb * QW
                acc_e = pso.tile([P, QW], F32, name=f"acc_e{hp}_{qb}", tag="acc")
                acc_o = pso.tile([P, QW], F32, name=f"acc_o{hp}_{qb}", tag="acc")
                exps = []
                np_prev = len(pend[3]) if pend is not None else 0
                for i, (kt, off, crossing) in enumerate(tiles):
                    ex = emit_scores(hp, q0, kt, off, crossing)
                    exps.append((kt, off, QW - off, ex))
                    if do_av and pend is not None:
                        # drain previous block's AVs at matching pace
                        lo = i * np_prev // len(tiles)
                        hi = (i + 1) * np_prev // len(tiles)
                        for j in range(lo, hi):
                            emit_av(pend, j)
                if pend is not None:
                    if do_av:
                        emit_norm(pend)
                    pend = None
                if do_av:
                    st = (hp, q0, (acc_e, acc_o), exps)
                    if pipeline:
                        pend = st
                    else:
                        for j in range(len(exps)):
                            emit_av(st, j)
                        emit_norm(st)
            if pend is not None and do_av:
                for j in range(len(pend[3])):
                    emit_av(pend, j)
                emit_norm(pend)
            if 2 in phases:
                emit_proj_group(range(done_qb * 4, done_qb * 4 + 4))
            else:
                emit_proj_group(range(TT))


def _get_compiled():
    global _COMPILED
    if _COMPILED is None:
        _COMPILED = _build_nc()
    return _COMPILED


def _make_in_maps(x, w_qkv, b_qkv, w_proj, all_bf16=True, qk_fp8=True):
    adt = _bf16np if all_bf16 else np.float32
    in_maps = []
    for c in range(N_CORES):
        b, g = c // 2, c % 2
        s = slice(g * GC, (g + 1) * GC)
        xTb = np.ascontiguousarray(x[b].T)
        w_qk = np.ascontiguousarray(
            np.concatenate([w_qkv[:, s], w_qkv[:, C + g * GC:C + (g + 1) * GC]],
                           axis=1))
        m = {
            "xT": xTb.astype(adt),
            "w_v": np.ascontiguousarray(
                w_qkv[:, 2 * C + g * GC:2 * C + (g + 1) * GC]).astype(adt),
            "b_qk": np.ascontiguousarray(
                np.concatenate([b_qkv[s], b_qkv[C + g * GC:C + (g + 1) * GC]])),
            "b_v": np.ascontiguousarray(b_qkv[2 * C + g * GC:2 * C + (g + 1) * GC]),
            "w_pr": np.ascontiguousarray(w_proj[g * GC:(g + 1) * GC, :]).astype(_bf16np),
        }
        if qk_fp8:
            m["xT8"] = xTb.astype(_fp8np)
            m["w_qk8"] = w_qk.astype(_fp8np)
        else:
            m["w_qk"] = w_qk.astype(adt)
        in_maps.append(m)
    return in_maps


_RUNNER = None


def _get_runner():
    """Compile once, cache the jitted shard_map executable across calls."""
    global _RUNNER
    if _RUNNER is not None:
        return _RUNNER
    import jax
    from jax.sharding import Mesh, PartitionSpec, NamedSharding
    from jax.experimental.shard_map import shard_map
    from concourse.bass2jax import (_bass_exec_p, install_neuronx_cc_hook,
                                    partition_id_tensor)

    nc = _get_compiled()
    install_neuronx_cc_hook()
    partition_name = nc.partition_id_tensor.name if nc.partition_id_tensor else None
    in_names, out_names, out_avals, zero_outs = [], [], [], []
    for alloc in nc.m.functions[0].allocations:
        if not isinstance(alloc, mybir.MemoryLocationSet):
            continue
        name = alloc.memorylocations[0].name
        if alloc.kind == "ExternalInput":
            if name != partition_name:
                in_names.append(name)
        elif alloc.kind == "ExternalOutput":
            out_names.append(name)
            out_avals.append(jax.core.ShapedArray(tuple(alloc.tensor_shape),
                                                  mybir.dt.np(alloc.dtype)))
            zero_outs.append(np.zeros(tuple(alloc.tensor_shape),
                                      mybir.dt.np(alloc.dtype)))
    all_in = list(in_names) + list(out_names)
    if partition_name:
        all_in.append(partition_name)

    def _body(*args):
        ops = list(args)
        if partition_name:
            ops.append(partition_id_tensor())
        return tuple(_bass_exec_p.bind(
            *ops, out_avals=tuple(out_avals), in_names=tuple(all_in),
            out_names=tuple(out_names), lowering_input_output_aliases=(),
            sim_require_finite=True, sim_require_nnan=True, nc=nc))

    devices = jax.devices()[:N_CORES]
    mesh = Mesh(np.asarray(devices), ("core",))
    sharded = jax.jit(shard_map(
        _body, mesh=mesh,
        in_specs=(PartitionSpec("core"),) * (len(in_names) + len(out_avals)),
        out_specs=(PartitionSpec("core"),) * len(out_avals), check_rep=False),
        keep_unused=True)
    sharding = NamedSharding(mesh, PartitionSpec("core"))
    _RUNNER = (sharded, in_names, zero_outs, sharding, out_avals, out_names)
    return _RUNNER


def _execute(in_maps):
    import jax
    sharded, in_names, zero_outs, sharding, out_avals, out_names = _get_runner()
    ci = [jax.device_put(
        np.concatenate([np.asarray(in_maps[c][n]) for c in range(N_CORES)], axis=0),
        sharding) for n in in_names]
    cz = [jax.device_put(np.zeros((N_CORES * z.shape[0], *z.shape[1:]), z.dtype),
                         sharding) for z in zero_outs]
    outs = sharded(*ci, *cz)
    yi = out_names.index("y")
    return np.asarray(outs[yi]).reshape(N_CORES, *out_avals[yi].shape)


def run(x, w_qkv, b_qkv, w_proj, b_proj, trace=False):
    in_maps = _make_in_maps(np.asarray(x, dtype=np.float32),
                            np.asarray(w_qkv, dtype=np.float32),
                            np.asarray(b_qkv, dtype=np.float32),
                            np.asarray(w_proj, dtype=np.float32))
    y8 = _execute(in_maps)
    out = np.empty((B, T, C), dtype=np.float32)
    bp = np.asarray(b_proj, dtype=np.float32)
    for b in range(B):
        out[b] = y8[2 * b] + y8[2 * b + 1] + bp
    return out


def kernel(x, w_qkv, b_qkv, w_proj, b_proj):
    return run(x, w_qkv, b_qkv, w_proj, b_proj)

